# revision 13
# baseline (speedup 1.0000x reference)
"""Multi-head attention TRN2 kernel (B=4, T=2048, C=1024, H=16, D=64).

Sharding: 8 cores = 4 batches x 2 head-halves. Core c handles batch c//2 and
heads (c%2)*8 .. (c%2)*8+8 (512 of the 1024 channel columns). Each core
computes a partial output projection; the host sums the two partials per
batch and adds the bp / bv rank-1 terms.

Per-core dataflow (all on one NeuronCore, no collectives):
  phase 1: qT = (Wq_c)^T x^T   [512, 2048]  (transposed layout, bias via DVE)
           kT likewise; v = x Wv_c [2048, 512] in natural layout, stored
           interleaved per head with a ones column appended ([128,8,65]).
  phase 2: per head pair (2c, 2c+1) sharing SBUF partition halves:
           scoresT[tk,tq] = k q^T / 8 on PE (K=64 row-tiled pairs),
           exp on ACT (no max subtraction; |scores| < ~8 so fp32 exp is
           safe), y^T_aug accumulated on PE with the ones-augmented v so
           row 64 is the softmax denominator. reciprocal on DVE,
           partition-broadcast on GPSIMD, normalize+bf16 on DVE.
  phase 3: out_partial = y^T(normalized)^T Wp_c on PE, fp32 DMA out.
"""

import os
import sys
from contextlib import ExitStack

import numpy as np

sys.path.insert(0, "/opt/trn_rl_repo")

import ml_dtypes  # noqa: E402

import concourse.bass as bass  # noqa: E402
import concourse.bacc as bacc  # noqa: E402
import concourse.mybir as mybir  # noqa: E402
import concourse.tile as tile  # noqa: E402
from concourse.bass_utils import run_bass_kernel_spmd  # noqa: E402

B, T, C, H, D = 4, 2048, 1024, 16, 64
HPC = 8          # heads per core
CC = HPC * D     # per-core channel columns = 512
NCORES = 8
BF16 = mybir.dt.bfloat16
F32 = mybir.dt.float32
BLK = 512        # tq block width
TKG = 2          # tk chunks per exp slab

_nc_cache = {}


def _build_nc():
    if "nc" in _nc_cache:
        return _nc_cache["nc"]
    nc = bacc.Bacc("TRN2", target_bir_lowering=False, debug=False)

    xT_d = nc.dram_tensor("xT", [C, T], BF16, kind="ExternalInput").ap()
    wq_d = nc.dram_tensor("wq", [C, CC], BF16, kind="ExternalInput").ap()
    wk_d = nc.dram_tensor("wk", [C, CC], BF16, kind="ExternalInput").ap()
    wv_d = nc.dram_tensor("wv", [C, CC], BF16, kind="ExternalInput").ap()
    wp_d = nc.dram_tensor("wp", [CC, C], BF16, kind="ExternalInput").ap()
    bq_d = nc.dram_tensor("bq2", [128, 4], F32, kind="ExternalInput").ap()
    bk_d = nc.dram_tensor("bk2", [128, 4], F32, kind="ExternalInput").ap()
    out_d = nc.dram_tensor("out", [T, C], F32, kind="ExternalOutput").ap()

    KC = C // 128    # 8 contraction chunks over C
    MC = CC // 128   # 4 column chunks of the per-core 512 cols
    NB = T // BLK    # 4 tq blocks
    TC = T // 128    # 16 tk chunks

    with tile.TileContext(nc) as tc, ExitStack() as ctx:
        p_wp = ctx.enter_context(tc.tile_pool(name="wp", bufs=MC))
        p_qk = ctx.enter_context(tc.tile_pool(name="qk", bufs=MC))
        p_kz = ctx.enter_context(tc.tile_pool(name="kz", bufs=2 * MC))
        p_v = ctx.enter_context(tc.tile_pool(name="v", bufs=TC))
        p_y = ctx.enter_context(tc.tile_pool(name="yn", bufs=MC))
        p_b = ctx.enter_context(tc.tile_pool(name="bias", bufs=1))
        # ---- phase 1: projections
        ph1 = ExitStack()
        p_x = ph1.enter_context(tc.tile_pool(name="x", bufs=KC))
        p_w = ph1.enter_context(tc.tile_pool(name="w", bufs=3 * KC))
        ps1 = ph1.enter_context(tc.tile_pool(name="ps1", bufs=4, space="PSUM"))
        # ---- load inputs
        xt = []
        for k in range(KC):
            t_ = p_x.tile([128, T], BF16, tag="xt")
            nc.sync.dma_start(t_[:], xT_d[k * 128:(k + 1) * 128, :])
            xt.append(t_)
        wq, wk, wv = [], [], []
        for name, dst, src in (("q", wq, wq_d), ("k", wk, wk_d), ("v", wv, wv_d)):
            for k in range(KC):
                t_ = p_w.tile([128, CC], BF16, tag=f"w{name}")
                nc.sync.dma_start(t_[:], src[k * 128:(k + 1) * 128, :])
                dst.append(t_)
        wp = []
        for k in range(MC):
            t_ = p_wp.tile([128, C], BF16, tag="wp")
            nc.sync.dma_start(t_[:], wp_d[k * 128:(k + 1) * 128, :])
            wp.append(t_)
        bq2 = p_b.tile([128, MC], F32, tag="bq")
        nc.sync.dma_start(bq2[:], bq_d[:])
        bk2 = p_b.tile([128, MC], F32, tag="bk")
        nc.sync.dma_start(bk2[:], bk_d[:])

        qt = []
        for m in range(MC):
            sb = p_qk.tile([128, T], BF16, tag="qt", name=f"qt{m}")
            qt.append(sb)
            for blk in range(NB):
                acc = ps1.tile([128, BLK], F32, tag="acc")
                for k in range(KC):
                    nc.tensor.matmul(
                        acc[:],
                        wq[k][:, m * 128:(m + 1) * 128],
                        xt[k][:, blk * BLK:(blk + 1) * BLK],
                        start=(k == 0), stop=(k == KC - 1),
                    )
                nc.vector.tensor_scalar_add(
                    sb[:, blk * BLK:(blk + 1) * BLK], acc[:], bq2[:, m:m + 1])

        # k: zero-padded per-head tiles; head h occupies its 64 native
        # partitions of chunk h//2, the other 64 rows stay zero so the
        # K=128 scores matmul adds nothing for the sibling head.
        kz = []
        for h in range(2 * MC):
            kzt = p_kz.tile([128, T], BF16, tag="kz", name=f"kz{h}")
            kz.append(kzt)
            # zero the sibling head's partition half
            zlo = 64 if h % 2 == 0 else 0
            nc.gpsimd.memset(kzt[zlo:zlo + 64, :], 0.0)
        for m in range(MC):
            for blk in range(NB):
                acc = ps1.tile([128, BLK], F32, tag="acc")
                for k in range(KC):
                    nc.tensor.matmul(
                        acc[:],
                        wk[k][:, m * 128:(m + 1) * 128],
                        xt[k][:, blk * BLK:(blk + 1) * BLK],
                        start=(k == 0), stop=(k == KC - 1),
                    )
                bc = slice(blk * BLK, (blk + 1) * BLK)
                nc.vector.tensor_scalar_add(
                    kz[2 * m][0:64, bc], acc[0:64, :], bk2[0:64, m:m + 1])
                nc.vector.tensor_scalar_add(
                    kz[2 * m + 1][64:128, bc], acc[64:128, :],
                    bk2[64:128, m:m + 1])

        vaug = []
        for t_ in range(TC):
            va = p_v.tile([128, HPC, D + 1], BF16, tag="va")
            vaug.append(va)
            nc.gpsimd.memset(va[:, :, D:D + 1], 1.0)
            acc = ps1.tile([128, CC], F32, tag="acc")
            for k in range(KC):
                nc.tensor.matmul(
                    acc[:],
                    xt[k][:, t_ * 128:(t_ + 1) * 128],
                    wv[k][:],
                    start=(k == 0), stop=(k == KC - 1),
                )
            nc.vector.tensor_copy(
                va[:, :, 0:D], acc[:].rearrange("p (h d) -> p h d", d=D))
        ph1.close()

        # ---- phase 2: attention per head pair
        ph2 = ExitStack()
        p_exp = ctx.enter_context(tc.tile_pool(name="exp", bufs=2))
        p_sm = ctx.enter_context(tc.tile_pool(name="sm", bufs=1))
        p_st = ctx.enter_context(tc.tile_pool(name="stage", bufs=2))
        ps_sc = ph2.enter_context(tc.tile_pool(name="psc", bufs=1, space="PSUM"))
        ps_y = ph2.enter_context(tc.tile_pool(name="psy", bufs=1, space="PSUM"))
        ytn = []
        for m in range(MC):
            yt_ = p_y.tile([128, T], BF16, tag="ytn", name=f"ytn{m}")
            ytn.append(yt_)

        ngrp = (TC + TKG - 1) // TKG
        groups = [list(range(g * TKG, min(TC, (g + 1) * TKG))) for g in range(ngrp)]

        for c in range(MC):          # head pair (2c, 2c+1)
            for blk in range(NB):
                bcols = slice(blk * BLK, (blk + 1) * BLK)
                y0 = ps_y.tile([D + 1, BLK], F32, tag="y0")
                y1 = ps_y.tile([D + 1, BLK], F32, tag="y1")
                for gi, grp in enumerate(groups):
                    s0 = ps_sc.tile([128, TKG, BLK], F32, tag="sc")
                    s1 = ps_sc.tile([128, TKG, BLK], F32, tag="sc", name="s1")
                    for j, tk in enumerate(grp):
                        tcols = slice(tk * 128, (tk + 1) * 128)
                        nc.tensor.matmul(
                            s0[:, j, :], kz[2 * c][:, tcols], qt[c][:, bcols],
                            start=True, stop=True)
                    for j, tk in enumerate(grp):
                        tcols = slice(tk * 128, (tk + 1) * 128)
                        nc.tensor.matmul(
                            s1[:, j, :], kz[2 * c + 1][:, tcols], qt[c][:, bcols],
                            start=True, stop=True)
                    e0 = p_exp.tile([128, TKG, BLK], BF16, tag="e0")
                    e1 = p_exp.tile([128, TKG, BLK], BF16, tag="e1")
                    n = len(grp)
                    nc.scalar.activation(
                        e0[:, 0:n, :], s0[:, 0:n, :],
                        mybir.ActivationFunctionType.Exp, scale=0.125)
                    nc.scalar.activation(
                        e1[:, 0:n, :], s1[:, 0:n, :],
                        mybir.ActivationFunctionType.Exp, scale=0.125)
                    for j, tk in enumerate(grp):
                        nc.tensor.matmul(
                            y0[:], vaug[tk][:, 2 * c, :], e0[:, j, :],
                            start=(tk == 0), stop=(tk == TC - 1))
                    for j, tk in enumerate(grp):
                        nc.tensor.matmul(
                            y1[:], vaug[tk][:, 2 * c + 1, :], e1[:, j, :],
                            start=(tk == 0), stop=(tk == TC - 1))
                for half, yy in ((0, y0), (1, y1)):
                    ys = p_sm.tile([D + 1, BLK], F32, tag=f"ys{half}")
                    nc.vector.tensor_copy(ys[:], yy[:])
                    rr = p_sm.tile([1, BLK], F32, tag=f"r{half}")
                    nc.vector.reciprocal(rr[:], ys[D:D + 1, :])
                    bb = p_sm.tile([D, BLK], F32, tag=f"b{half}")
                    nc.gpsimd.partition_broadcast(bb[:], rr[:])
                    nc.vector.tensor_mul(
                        ytn[c][half * 64:half * 64 + 64, bcols],
                        ys[0:D, :], bb[:])

        ph2.close()

        # ---- phase 3: output projection (partial; host adds the pair)
        ps3 = ctx.enter_context(tc.tile_pool(name="ps3", bufs=2, space="PSUM"))
        for tq in range(T // 128):
            rows = slice(tq * 128, (tq + 1) * 128)
            po = ps3.tile([128, 2, BLK], F32, tag="po")
            for half in range(2):
                for k in range(MC):
                    nc.tensor.matmul(
                        po[:, half, :],
                        ytn[k][:, rows],
                        wp[k][:, half * BLK:(half + 1) * BLK],
                        start=(k == 0), stop=(k == MC - 1))
            st = p_st.tile([128, 2, BLK], F32, tag="st")
            nc.vector.tensor_copy(st[:], po[:])
            nc.sync.dma_start(
                out_d[rows, :].rearrange("p (a b) -> p a b", a=2), st[:])

    nc.compile()
    _nc_cache["nc"] = nc
    return nc


def prepare_in_maps(x, Wq, bq, Wk, bk, Wv, bv, Wp, bp):
    x = np.asarray(x, dtype=np.float32)
    Wq, bq = np.asarray(Wq, np.float32), np.asarray(bq, np.float32)
    Wk, bk = np.asarray(Wk, np.float32), np.asarray(bk, np.float32)
    Wv = np.asarray(Wv, np.float32)
    Wp = np.asarray(Wp, np.float32)
    bf = ml_dtypes.bfloat16

    in_maps = []
    for c in range(NCORES):
        b, half = divmod(c, 2)
        cols = slice(half * CC, (half + 1) * CC)
        in_maps.append({
            "xT": np.ascontiguousarray(x[b].T).astype(bf),
            "wq": np.ascontiguousarray(Wq[:, cols]).astype(bf),
            "wk": np.ascontiguousarray(Wk[:, cols]).astype(bf),
            "wv": np.ascontiguousarray(Wv[:, cols]).astype(bf),
            "wp": np.ascontiguousarray(Wp[cols, :]).astype(bf),
            "bq2": np.ascontiguousarray(bq[cols].reshape(4, 128).T),
            "bk2": np.ascontiguousarray(bk[cols].reshape(4, 128).T),
        })
    return in_maps


def combine(results, Wv, bv, Wp, bp):
    bv = np.asarray(bv, np.float32)
    Wp = np.asarray(Wp, np.float32)
    bp = np.asarray(bp, np.float32)
    out = np.zeros((B, T, C), np.float32)
    for c in range(NCORES):
        b, half = divmod(c, 2)
        cols = slice(half * CC, (half + 1) * CC)
        out[b] += results[c]["out"]
        # bv enters y as att@1 * bv = bv per row (softmax rows sum to 1)
        out[b] += bv[cols] @ Wp[cols, :]
    out += bp
    return out


def kernel(x, Wq, bq, Wk, bk, Wv, bv, Wp, bp):
    in_maps = prepare_in_maps(x, Wq, bq, Wk, bk, Wv, bv, Wp, bp)
    nc = _build_nc()
    res = run_bass_kernel_spmd(nc, in_maps, list(range(NCORES))).results
    return combine(res, Wv, bv, Wp, bp)


# revision 15
# speedup vs baseline: 2.0879x; 2.0879x over previous
"""Multi-head attention TRN2 kernel (B=4, T=2048, C=1024, H=16, D=64).

Sharding: 8 cores = 4 batches x 2 head-halves. Core c handles batch c//2 and
heads (c%2)*8 .. (c%2)*8+8 (512 of the 1024 channel columns). Each core
computes a partial output projection; the host sums the two partials per
batch and adds the bp / bv rank-1 terms.

Per-core dataflow (all on one NeuronCore, no collectives):
  phase 1: qT = (Wq_c)^T x^T   [512, 2048]  (transposed layout, bias via DVE)
           kT likewise; v = x Wv_c [2048, 512] in natural layout, stored
           interleaved per head with a ones column appended ([128,8,65]).
  phase 2: per head pair (2c, 2c+1) sharing SBUF partition halves:
           scoresT[tk,tq] = k q^T / 8 on PE (K=64 row-tiled pairs),
           exp on ACT (no max subtraction; |scores| < ~8 so fp32 exp is
           safe), y^T_aug accumulated on PE with the ones-augmented v so
           row 64 is the softmax denominator. reciprocal on DVE,
           partition-broadcast on GPSIMD, normalize+bf16 on DVE.
  phase 3: out_partial = y^T(normalized)^T Wp_c on PE, fp32 DMA out.
"""

import os
import sys
from contextlib import ExitStack

import numpy as np

sys.path.insert(0, "/opt/trn_rl_repo")

import ml_dtypes  # noqa: E402

import concourse.bass as bass  # noqa: E402
import concourse.bacc as bacc  # noqa: E402
import concourse.mybir as mybir  # noqa: E402
import concourse.tile as tile  # noqa: E402
from concourse.bass_utils import run_bass_kernel_spmd  # noqa: E402

B, T, C, H, D = 4, 2048, 1024, 16, 64
HPC = 8          # heads per core
CC = HPC * D     # per-core channel columns = 512
NCORES = 8
BF16 = mybir.dt.bfloat16
F32 = mybir.dt.float32
BLK = 512        # tq block width
TKG = 3          # tk chunks per exp slab

_nc_cache = {}


def _build_nc():
    if "nc" in _nc_cache:
        return _nc_cache["nc"]
    nc = bacc.Bacc("TRN2", target_bir_lowering=False, debug=False)

    xT_d = nc.dram_tensor("xT", [C, T], BF16, kind="ExternalInput").ap()
    wq_d = nc.dram_tensor("wq", [C, CC], BF16, kind="ExternalInput").ap()
    wk_d = nc.dram_tensor("wk", [C, CC], BF16, kind="ExternalInput").ap()
    wv_d = nc.dram_tensor("wv", [C, CC], BF16, kind="ExternalInput").ap()
    wp_d = nc.dram_tensor("wp", [CC, C], BF16, kind="ExternalInput").ap()
    bq_d = nc.dram_tensor("bq2", [128, 4], F32, kind="ExternalInput").ap()
    bk_d = nc.dram_tensor("bk2", [128, 4], F32, kind="ExternalInput").ap()
    out_d = nc.dram_tensor("out", [T, C], F32, kind="ExternalOutput").ap()

    KC = C // 128    # 8 contraction chunks over C
    MC = CC // 128   # 4 column chunks of the per-core 512 cols
    NB = T // BLK    # 4 tq blocks
    TC = T // 128    # 16 tk chunks

    with tile.TileContext(nc) as tc, ExitStack() as ctx:
        p_wp = ctx.enter_context(tc.tile_pool(name="wp", bufs=MC))
        p_qk = ctx.enter_context(tc.tile_pool(name="qk", bufs=MC))
        p_kz = ctx.enter_context(tc.tile_pool(name="kz", bufs=2 * MC))
        p_v = ctx.enter_context(tc.tile_pool(name="v", bufs=TC))
        p_y = ctx.enter_context(tc.tile_pool(name="yn", bufs=MC))
        p_b = ctx.enter_context(tc.tile_pool(name="bias", bufs=1))
        # ---- phase 1: projections
        ph1 = ExitStack()
        p_x = ph1.enter_context(tc.tile_pool(name="x", bufs=KC))
        p_w = ph1.enter_context(tc.tile_pool(name="w", bufs=3 * KC))
        ps1 = ph1.enter_context(tc.tile_pool(name="ps1", bufs=4, space="PSUM"))
        # ---- load inputs
        xt = []
        for k in range(KC):
            t_ = p_x.tile([128, T], BF16, tag="xt")
            nc.sync.dma_start(t_[:], xT_d[k * 128:(k + 1) * 128, :])
            xt.append(t_)
        wq, wk, wv = [], [], []
        for name, dst, src in (("q", wq, wq_d), ("k", wk, wk_d), ("v", wv, wv_d)):
            for k in range(KC):
                t_ = p_w.tile([128, CC], BF16, tag=f"w{name}")
                nc.sync.dma_start(t_[:], src[k * 128:(k + 1) * 128, :])
                dst.append(t_)
        wp = []
        for k in range(MC):
            t_ = p_wp.tile([128, C], BF16, tag="wp")
            nc.sync.dma_start(t_[:], wp_d[k * 128:(k + 1) * 128, :])
            wp.append(t_)
        bq2 = p_b.tile([128, MC], F32, tag="bq")
        nc.sync.dma_start(bq2[:], bq_d[:])
        bk2 = p_b.tile([128, MC], F32, tag="bk")
        nc.sync.dma_start(bk2[:], bk_d[:])

        qt = []
        for m in range(MC):
            sb = p_qk.tile([128, T], BF16, tag="qt", name=f"qt{m}")
            qt.append(sb)
            for blk in range(NB):
                acc = ps1.tile([128, BLK], F32, tag="acc")
                for k in range(KC):
                    nc.tensor.matmul(
                        acc[:],
                        wq[k][:, m * 128:(m + 1) * 128],
                        xt[k][:, blk * BLK:(blk + 1) * BLK],
                        start=(k == 0), stop=(k == KC - 1),
                    )
                nc.vector.tensor_scalar_add(
                    sb[:, blk * BLK:(blk + 1) * BLK], acc[:], bq2[:, m:m + 1])

        # k: zero-padded per-head tiles; head h occupies its 64 native
        # partitions of chunk h//2, the other 64 rows stay zero so the
        # K=128 scores matmul adds nothing for the sibling head.
        kz = []
        for h in range(2 * MC):
            kzt = p_kz.tile([128, T], BF16, tag="kz", name=f"kz{h}")
            kz.append(kzt)
            # zero the sibling head's partition half
            zlo = 64 if h % 2 == 0 else 0
            nc.gpsimd.memset(kzt[zlo:zlo + 64, :], 0.0)
        for m in range(MC):
            for blk in range(NB):
                acc = ps1.tile([128, BLK], F32, tag="acc")
                for k in range(KC):
                    nc.tensor.matmul(
                        acc[:],
                        wk[k][:, m * 128:(m + 1) * 128],
                        xt[k][:, blk * BLK:(blk + 1) * BLK],
                        start=(k == 0), stop=(k == KC - 1),
                    )
                bc = slice(blk * BLK, (blk + 1) * BLK)
                nc.vector.tensor_scalar_add(
                    kz[2 * m][0:64, bc], acc[0:64, :], bk2[0:64, m:m + 1])
                nc.vector.tensor_scalar_add(
                    kz[2 * m + 1][64:128, bc], acc[64:128, :],
                    bk2[64:128, m:m + 1])

        vaug = []
        for t_ in range(TC):
            va = p_v.tile([128, HPC, D + 1], BF16, tag="va")
            vaug.append(va)
            nc.gpsimd.memset(va[:, :, D:D + 1], 1.0)
            acc = ps1.tile([128, CC], F32, tag="acc")
            for k in range(KC):
                nc.tensor.matmul(
                    acc[:],
                    xt[k][:, t_ * 128:(t_ + 1) * 128],
                    wv[k][:],
                    start=(k == 0), stop=(k == KC - 1),
                )
            nc.vector.tensor_copy(
                va[:, :, 0:D], acc[:].rearrange("p (h d) -> p h d", d=D))
        ph1.close()

        # ---- phase 2: attention per head pair
        ph2 = ExitStack()
        p_exp = ctx.enter_context(tc.tile_pool(name="exp", bufs=2))
        p_sm = ctx.enter_context(tc.tile_pool(name="sm", bufs=1))
        p_st = ctx.enter_context(tc.tile_pool(name="stage", bufs=2))
        ps_sc = ph2.enter_context(tc.tile_pool(name="psc", bufs=1, space="PSUM"))
        ps_y = ph2.enter_context(tc.tile_pool(name="psy", bufs=1, space="PSUM"))
        ytn = []
        for m in range(MC):
            yt_ = p_y.tile([128, T], BF16, tag="ytn", name=f"ytn{m}")
            ytn.append(yt_)

        ngrp = (TC + TKG - 1) // TKG
        groups = [list(range(g * TKG, min(TC, (g + 1) * TKG))) for g in range(ngrp)]

        for c in range(MC):          # head pair (2c, 2c+1)
            for blk in range(NB):
                bcols = slice(blk * BLK, (blk + 1) * BLK)
                y0 = ps_y.tile([D + 1, BLK], F32, tag="y0")
                y1 = ps_y.tile([D + 1, BLK], F32, tag="y1")
                for gi, grp in enumerate(groups):
                    s0 = ps_sc.tile([128, TKG, BLK], F32, tag="s0")
                    s1 = ps_sc.tile([128, TKG, BLK], F32, tag="s1")
                    for j, tk in enumerate(grp):
                        tcols = slice(tk * 128, (tk + 1) * 128)
                        nc.tensor.matmul(
                            s0[:, j, :], kz[2 * c][:, tcols], qt[c][:, bcols],
                            start=True, stop=True)
                    for j, tk in enumerate(grp):
                        tcols = slice(tk * 128, (tk + 1) * 128)
                        nc.tensor.matmul(
                            s1[:, j, :], kz[2 * c + 1][:, tcols], qt[c][:, bcols],
                            start=True, stop=True)
                    e0 = p_exp.tile([128, TKG, BLK], BF16, tag="e0")
                    e1 = p_exp.tile([128, TKG, BLK], BF16, tag="e1")
                    n = len(grp)
                    nc.scalar.activation(
                        e0[:, 0:n, :], s0[:, 0:n, :],
                        mybir.ActivationFunctionType.Exp, scale=0.125)
                    nc.scalar.activation(
                        e1[:, 0:n, :], s1[:, 0:n, :],
                        mybir.ActivationFunctionType.Exp, scale=0.125)
                    for j, tk in enumerate(grp):
                        nc.tensor.matmul(
                            y0[:], vaug[tk][:, 2 * c, :], e0[:, j, :],
                            start=(tk == 0), stop=(tk == TC - 1))
                    for j, tk in enumerate(grp):
                        nc.tensor.matmul(
                            y1[:], vaug[tk][:, 2 * c + 1, :], e1[:, j, :],
                            start=(tk == 0), stop=(tk == TC - 1))
                for half, yy in ((0, y0), (1, y1)):
                    ys = p_sm.tile([D + 1, BLK], F32, tag=f"ys{half}")
                    nc.vector.tensor_copy(ys[:], yy[:])
                    rr = p_sm.tile([1, BLK], F32, tag=f"r{half}")
                    nc.vector.reciprocal(rr[:], ys[D:D + 1, :])
                    bb = p_sm.tile([D, BLK], F32, tag=f"b{half}")
                    nc.gpsimd.partition_broadcast(bb[:], rr[:])
                    nc.vector.tensor_mul(
                        ytn[c][half * 64:half * 64 + 64, bcols],
                        ys[0:D, :], bb[:])

        ph2.close()

        # ---- phase 3: output projection (partial; host adds the pair)
        ps3 = ctx.enter_context(tc.tile_pool(name="ps3", bufs=2, space="PSUM"))
        for tq in range(T // 128):
            rows = slice(tq * 128, (tq + 1) * 128)
            po = ps3.tile([128, 2, BLK], F32, tag="po")
            for half in range(2):
                for k in range(MC):
                    nc.tensor.matmul(
                        po[:, half, :],
                        ytn[k][:, rows],
                        wp[k][:, half * BLK:(half + 1) * BLK],
                        start=(k == 0), stop=(k == MC - 1))
            st = p_st.tile([128, 2, BLK], F32, tag="st")
            nc.vector.tensor_copy(st[:], po[:])
            nc.sync.dma_start(
                out_d[rows, :].rearrange("p (a b) -> p a b", a=2), st[:])

    nc.compile()
    _nc_cache["nc"] = nc
    return nc


def prepare_in_maps(x, Wq, bq, Wk, bk, Wv, bv, Wp, bp):
    x = np.asarray(x, dtype=np.float32)
    Wq, bq = np.asarray(Wq, np.float32), np.asarray(bq, np.float32)
    Wk, bk = np.asarray(Wk, np.float32), np.asarray(bk, np.float32)
    Wv = np.asarray(Wv, np.float32)
    Wp = np.asarray(Wp, np.float32)
    bf = ml_dtypes.bfloat16

    in_maps = []
    for c in range(NCORES):
        b, half = divmod(c, 2)
        cols = slice(half * CC, (half + 1) * CC)
        in_maps.append({
            "xT": np.ascontiguousarray(x[b].T).astype(bf),
            "wq": np.ascontiguousarray(Wq[:, cols]).astype(bf),
            "wk": np.ascontiguousarray(Wk[:, cols]).astype(bf),
            "wv": np.ascontiguousarray(Wv[:, cols]).astype(bf),
            "wp": np.ascontiguousarray(Wp[cols, :]).astype(bf),
            "bq2": np.ascontiguousarray(bq[cols].reshape(4, 128).T),
            "bk2": np.ascontiguousarray(bk[cols].reshape(4, 128).T),
        })
    return in_maps


def combine(results, Wv, bv, Wp, bp):
    bv = np.asarray(bv, np.float32)
    Wp = np.asarray(Wp, np.float32)
    bp = np.asarray(bp, np.float32)
    out = np.zeros((B, T, C), np.float32)
    for c in range(NCORES):
        b, half = divmod(c, 2)
        cols = slice(half * CC, (half + 1) * CC)
        out[b] += results[c]["out"]
        # bv enters y as att@1 * bv = bv per row (softmax rows sum to 1)
        out[b] += bv[cols] @ Wp[cols, :]
    out += bp
    return out


def kernel(x, Wq, bq, Wk, bk, Wv, bv, Wp, bp):
    in_maps = prepare_in_maps(x, Wq, bq, Wk, bk, Wv, bv, Wp, bp)
    nc = _build_nc()
    res = run_bass_kernel_spmd(nc, in_maps, list(range(NCORES))).results
    return combine(res, Wv, bv, Wp, bp)
